# revision 4
# baseline (speedup 1.0000x reference)
"""DeepSeek-V2 MoE layer (T=2048, H=2048, I=1408, E=8, top-2) on 8 TRN2 cores.

Strategy: expert parallelism. The router (67 MFLOP, 0.06% of total work) runs
on the host to produce the token->expert dispatch; each NeuronCore runs one
expert's gate/up/down GEMMs over the tokens routed to it (padded to a fixed
capacity C), with the top-2 combine weight folded into the output. The host
scatter-adds the per-expert outputs back into the full [T, H] output.

Matmuls run as fp32r (TF32-like, 11 mantissa bits, full PE rate). Inputs are
pre-rounded to the fp32r-representable subset on the host so tiles can be
DMA'd directly (the BIR verifier requires fp32r matmul operands to come from
a rounding producer or be declared fp32r end-to-end).
"""
import sys

_TRN = "/opt/trn_rl_repo"
if _TRN not in sys.path:
    sys.path.insert(0, _TRN)

import numpy as np

import concourse.bacc as bacc
import concourse.mybir as mybir
import concourse.tile as tile
from concourse import bass_utils

T, H, I, E = 2048, 2048, 1408, 8
C = 640                       # per-expert token capacity (actual max count: 545)
NT, NI, NCT = H // 128, I // 128, C // 128   # 16, 11, 5
F32, F32R = mybir.dt.float32, mybir.dt.float32r
CH = C // 2                   # matmul free-dim split for phase A (320 >= 256)

_CACHE = {}


def _round_f32r(x: np.ndarray) -> np.ndarray:
    """Round fp32 to the fp32r-representable subset (RNE to 11 mantissa bits)."""
    u = np.ascontiguousarray(x, dtype=np.float32).view(np.uint32).astype(np.uint64)
    u = u + 0x7FF + ((u >> 12) & 1)
    return (u & np.uint64(0xFFFFF000)).astype(np.uint32).view(np.float32)


def _build():
    nc = bacc.Bacc("TRN2", target_bir_lowering=False, debug=False, num_devices=8)
    xt_d = nc.dram_tensor("xt", [H, C], F32R, kind="ExternalInput").ap()
    wg_d = nc.dram_tensor("wg", [NI, 128, H], F32R, kind="ExternalInput").ap()
    wu_d = nc.dram_tensor("wu", [NI, 128, H], F32R, kind="ExternalInput").ap()
    wd_d = nc.dram_tensor("wd", [I, H], F32R, kind="ExternalInput").ap()
    cmb_d = nc.dram_tensor("cmb", [C, 1], F32, kind="ExternalInput").ap()
    y_d = nc.dram_tensor("y", [C, H], F32, kind="ExternalOutput").ap()

    with tile.TileContext(nc) as tc:
        with (
            tc.tile_pool(name="xtp", bufs=1) as xtp,
            tc.tile_pool(name="wp", bufs=2) as wp,
            tc.tile_pool(name="htp", bufs=NI) as htp,
            tc.tile_pool(name="wdp", bufs=NI) as wdp,
            tc.tile_pool(name="mp", bufs=2) as mp,
            tc.tile_pool(name="op", bufs=1) as op,
        ):
            # Resident loads: x^T (tiled by h-block) and combine weights.
            xt = xtp.tile([128, NT, C], F32R, tag="xt")
            for t in range(NT):
                nc.sync.dma_start(xt[:, t, :], xt_d[t * 128:(t + 1) * 128, :])
            cmb = xtp.tile([128, NCT], F32, tag="cmb")
            for c in range(NCT):
                nc.sync.dma_start(cmb[:, c:c + 1], cmb_d[c * 128:(c + 1) * 128, :])

            wd_tiles = []
            ht_tiles = []

            # Phase A: hT[i] = silu(Wg[:,i]^T x^T) * (Wu[:,i]^T x^T), [128, C]
            # Each matmul output must stay inside one 2KB PSUM bank and
            # start=True clears the whole bank, so the C=640 free dim is
            # split into two single-bank tiles of 384 and 256 columns
            # (both >=256 keeps fp32r at full rate).
            SPLITS = ((0, 384), (384, 256))
            with tc.tile_pool(name="psA", bufs=2, space="PSUM") as psA:
                for i in range(NI):
                    wgt = wp.tile([128, H], F32R, tag="wg")
                    wut = wp.tile([128, H], F32R, tag="wu")
                    nc.sync.dma_start(wgt[:], wg_d[i])
                    nc.sync.dma_start(wut[:], wu_d[i])
                    pg = [psA.tile([128, w], F32, tag=f"pg{k}", name=f"pg{k}_{i}")
                          for k, (_, w) in enumerate(SPLITS)]
                    pu = [psA.tile([128, w], F32, tag=f"pu{k}", name=f"pu{k}_{i}")
                          for k, (_, w) in enumerate(SPLITS)]
                    for t in range(NT):
                        s = slice(t * 128, (t + 1) * 128)
                        for k, (lo, w) in enumerate(SPLITS):
                            nc.tensor.matmul(pg[k][:], wgt[:, s],
                                             xt[:, t, lo:lo + w],
                                             start=(t == 0), stop=(t == NT - 1))
                    for t in range(NT):
                        s = slice(t * 128, (t + 1) * 128)
                        for k, (lo, w) in enumerate(SPLITS):
                            nc.tensor.matmul(pu[k][:], wut[:, s],
                                             xt[:, t, lo:lo + w],
                                             start=(t == 0), stop=(t == NT - 1))
                    tmp = mp.tile([128, C], F32, tag="tmp")
                    ht = htp.tile([128, C], F32R, tag="ht")
                    for k, (lo, w) in enumerate(SPLITS):
                        nc.scalar.activation(tmp[:, lo:lo + w], pg[k][:],
                                             mybir.ActivationFunctionType.Silu)
                        nc.vector.tensor_mul(ht[:, lo:lo + w], tmp[:, lo:lo + w],
                                             pu[k][:])
                    ht_tiles.append(ht)

                    # Trickle-in the down-proj weights during phase A.
                    wdt = wdp.tile([128, H], F32R, tag="wd")
                    nc.sync.dma_start(wdt[:], wd_d[i * 128:(i + 1) * 128, :])
                    wd_tiles.append(wdt)

            # Phase B: y[c-block] = sum_i hT[i][:, c-block]^T @ Wd[i], scaled
            # by the per-token combine weight.
            with tc.tile_pool(name="psB", bufs=2, space="PSUM") as psB:
                for c in range(NCT):
                    po = psB.tile([128, H], F32, tag="po")
                    cs = slice(c * 128, (c + 1) * 128)
                    for i in range(NI):
                        for n in range(4):
                            ns = slice(n * 512, (n + 1) * 512)
                            nc.tensor.matmul(po[:, ns], ht_tiles[i][:, cs],
                                             wd_tiles[i][:, ns],
                                             start=(i == 0), stop=(i == NI - 1))
                    ot = op.tile([128, H], F32, tag="ot")
                    nc.vector.tensor_scalar_mul(ot[:], po[:], cmb[:, c:c + 1])
                    nc.sync.dma_start(y_d[cs, :], ot[:])

    nc.compile()
    return nc


def _route(X: np.ndarray, Wr: np.ndarray):
    """Host router: top-2 of softmax(X @ Wr), renormalized over the top-2."""
    logits = X.astype(np.float64) @ Wr.astype(np.float64)
    order = np.argsort(-logits, axis=1)
    top1, top2 = order[:, 0], order[:, 1]
    rows = np.arange(T)
    l1, l2 = logits[rows, top1], logits[rows, top2]
    e21 = np.exp(l2 - l1)
    w1 = 1.0 / (1.0 + e21)
    w2 = e21 / (1.0 + e21)
    return top1, top2, w1.astype(np.float32), w2.astype(np.float32)


def _reference_numpy(hidden_states, w_router, w_gate, w_up, w_down):
    X = np.asarray(hidden_states, np.float32)
    top1, top2, w1, w2 = _route(X, np.asarray(w_router, np.float32))
    out = np.zeros((T, H), np.float32)
    for e in range(E):
        sel = np.where((top1 == e) | (top2 == e))[0]
        if len(sel) == 0:
            continue
        w = np.where(top1[sel] == e, w1[sel], w2[sel])[:, None]
        x = X[sel]
        h = (x @ w_gate[e])
        h = (h / (1.0 + np.exp(-h))) * (x @ w_up[e]) * w
        out[sel] += h @ w_down[e]
    return out


def kernel(hidden_states, w_router, w_gate, w_up, w_down):
    X = np.ascontiguousarray(hidden_states, dtype=np.float32)
    Wr = np.ascontiguousarray(w_router, dtype=np.float32)
    Wg = np.ascontiguousarray(w_gate, dtype=np.float32)
    Wu = np.ascontiguousarray(w_up, dtype=np.float32)
    Wd = np.ascontiguousarray(w_down, dtype=np.float32)

    top1, top2, w1, w2 = _route(X, Wr)
    sels, wts = [], []
    for e in range(E):
        sel = np.where((top1 == e) | (top2 == e))[0]
        sels.append(sel)
        wts.append(np.where(top1[sel] == e, w1[sel], w2[sel]))
    if max(len(s) for s in sels) > C:
        # Capacity overflow (cannot happen for the reference input
        # distribution); fall back to a host implementation.
        return _reference_numpy(X, Wr, Wg, Wu, Wd)

    if "nc" not in _CACHE:
        _CACHE["nc"] = _build()
    nc = _CACHE["nc"]

    Xr = _round_f32r(X)
    in_maps = []
    for e in range(E):
        sel, w = sels[e], wts[e]
        n = len(sel)
        xt = np.zeros((H, C), np.float32)
        xt[:, :n] = Xr[sel].T
        cmb = np.zeros((C, 1), np.float32)
        cmb[:n, 0] = w
        wg_sw = (_round_f32r(Wg[e]).reshape(NT, 128, NI, 128)
                 .transpose(2, 1, 0, 3).reshape(NI, 128, H))
        wu_sw = (_round_f32r(Wu[e]).reshape(NT, 128, NI, 128)
                 .transpose(2, 1, 0, 3).reshape(NI, 128, H))
        wd_r = _round_f32r(Wd[e])
        in_maps.append({
            "xt": np.ascontiguousarray(xt),
            "wg": np.ascontiguousarray(wg_sw),
            "wu": np.ascontiguousarray(wu_sw),
            "wd": wd_r,
            "cmb": cmb,
        })

    res = bass_utils.run_bass_kernel_spmd(nc, in_maps, list(range(E)))

    out = np.zeros((T, H), np.float32)
    for e in range(E):
        sel = sels[e]
        out[sel] += res.results[e]["y"][:len(sel)]
    return out


# revision 7
# speedup vs baseline: 1.0492x; 1.0492x over previous
"""DeepSeek-V2 MoE layer (T=2048, H=2048, I=1408, E=8, top-2) on 8 TRN2 cores.

Strategy: expert parallelism. The router (67 MFLOP, 0.06% of total work) runs
on the host to produce the token->expert dispatch; each NeuronCore runs one
expert's gate/up/down GEMMs over the tokens routed to it (padded to a fixed
capacity C), with the top-2 combine weight folded into the output. The host
scatter-adds the per-expert outputs back into the full [T, H] output.

Matmuls run as fp32r (TF32-like, 11 mantissa bits, full PE rate). Inputs are
pre-rounded to the fp32r-representable subset on the host so tiles can be
DMA'd directly (the BIR verifier requires fp32r matmul operands to come from
a rounding producer or be declared fp32r end-to-end).
"""
import sys

_TRN = "/opt/trn_rl_repo"
if _TRN not in sys.path:
    sys.path.insert(0, _TRN)

import numpy as np

import concourse.bacc as bacc
import concourse.mybir as mybir
import concourse.tile as tile
from concourse import bass_utils

T, H, I, E = 2048, 2048, 1408, 8
C = 640                       # per-expert token capacity (actual max count: 545)
NT, NI, NCT = H // 128, I // 128, C // 128   # 16, 11, 5
F32, F32R = mybir.dt.float32, mybir.dt.float32r
CH = C // 2                   # matmul free-dim split for phase A (320 >= 256)

_CACHE = {}


def _round_f32r(x: np.ndarray) -> np.ndarray:
    """Round fp32 to the fp32r-representable subset (RNE to 11 mantissa bits)."""
    u = np.ascontiguousarray(x, dtype=np.float32).view(np.uint32).astype(np.uint64)
    u = u + 0x7FF + ((u >> 12) & 1)
    return (u & np.uint64(0xFFFFF000)).astype(np.uint32).view(np.float32)


def _build():
    nc = bacc.Bacc("TRN2", target_bir_lowering=False, debug=False, num_devices=8)
    xt_d = nc.dram_tensor("xt", [H, C], F32R, kind="ExternalInput").ap()
    wg_d = nc.dram_tensor("wg", [NI, 128, H], F32R, kind="ExternalInput").ap()
    wu_d = nc.dram_tensor("wu", [NI, 128, H], F32R, kind="ExternalInput").ap()
    wd_d = nc.dram_tensor("wd", [I, H], F32R, kind="ExternalInput").ap()
    cmb_d = nc.dram_tensor("cmb", [C, 1], F32, kind="ExternalInput").ap()
    y_d = nc.dram_tensor("y", [C, H], F32, kind="ExternalOutput").ap()

    with tile.TileContext(nc) as tc:
        with (
            tc.tile_pool(name="xtp", bufs=1) as xtp,
            tc.tile_pool(name="wp", bufs=2) as wp,
            tc.tile_pool(name="htp", bufs=NI) as htp,
            tc.tile_pool(name="wdp", bufs=NI) as wdp,
            tc.tile_pool(name="mp", bufs=2) as mp,
            tc.tile_pool(name="op", bufs=1) as op,
        ):
            # Front loads. Order matters: the first phase-A iteration's
            # weights go first so the PE can start ~6us in; x^T tiles
            # stream next (consumed progressively by the t-loop); cmb is
            # only needed in phase B.
            wgt0 = wp.tile([128, H], F32R, tag="wg", name="wgt0")
            wut0 = wp.tile([128, H], F32R, tag="wu", name="wut0")
            nc.sync.dma_start(wgt0[:], wg_d[0])
            nc.sync.dma_start(wut0[:], wu_d[0])
            xt = xtp.tile([128, NT, C], F32R, tag="xt")
            for t in range(NT):
                nc.sync.dma_start(xt[:, t, :], xt_d[t * 128:(t + 1) * 128, :])
            cmb = xtp.tile([128, NCT], F32, tag="cmb")
            for c in range(NCT):
                nc.sync.dma_start(cmb[:, c:c + 1], cmb_d[c * 128:(c + 1) * 128, :])

            wd_tiles = []
            ht_tiles = []

            # Phase A: hT[i] = silu(Wg[:,i]^T x^T) * (Wu[:,i]^T x^T), [128, C]
            # Each matmul output must stay inside one 2KB PSUM bank and
            # start=True clears the whole bank, so the C=640 free dim is
            # split into two single-bank tiles of 384 and 256 columns
            # (both >=256 keeps fp32r at full rate).
            SPLITS = ((0, 384), (384, 256))
            with tc.tile_pool(name="psA", bufs=2, space="PSUM") as psA:
                for i in range(NI):
                    if i == 0:
                        wgt, wut = wgt0, wut0
                    else:
                        wgt = wp.tile([128, H], F32R, tag="wg")
                        wut = wp.tile([128, H], F32R, tag="wu")
                        nc.sync.dma_start(wgt[:], wg_d[i])
                        nc.sync.dma_start(wut[:], wu_d[i])
                    pg = [psA.tile([128, w], F32, tag=f"pg{k}", name=f"pg{k}_{i}")
                          for k, (_, w) in enumerate(SPLITS)]
                    pu = [psA.tile([128, w], F32, tag=f"pu{k}", name=f"pu{k}_{i}")
                          for k, (_, w) in enumerate(SPLITS)]
                    for t in range(NT):
                        s = slice(t * 128, (t + 1) * 128)
                        for k, (lo, w) in enumerate(SPLITS):
                            nc.tensor.matmul(pg[k][:], wgt[:, s],
                                             xt[:, t, lo:lo + w],
                                             start=(t == 0), stop=(t == NT - 1))
                    for t in range(NT):
                        s = slice(t * 128, (t + 1) * 128)
                        for k, (lo, w) in enumerate(SPLITS):
                            nc.tensor.matmul(pu[k][:], wut[:, s],
                                             xt[:, t, lo:lo + w],
                                             start=(t == 0), stop=(t == NT - 1))
                    tmp = mp.tile([128, C], F32, tag="tmp")
                    ht = htp.tile([128, C], F32R, tag="ht")
                    for k, (lo, w) in enumerate(SPLITS):
                        nc.scalar.activation(tmp[:, lo:lo + w], pg[k][:],
                                             mybir.ActivationFunctionType.Silu)
                        nc.vector.tensor_mul(ht[:, lo:lo + w], tmp[:, lo:lo + w],
                                             pu[k][:])
                    ht_tiles.append(ht)

                    # Trickle-in the down-proj weights during phase A.
                    wdt = wdp.tile([128, H], F32R, tag="wd")
                    nc.sync.dma_start(wdt[:], wd_d[i * 128:(i + 1) * 128, :])
                    wd_tiles.append(wdt)

            # Phase B: y[c-block] = sum_i hT[i][:, c-block]^T @ Wd[i], scaled
            # by the per-token combine weight.
            with tc.tile_pool(name="psB", bufs=2, space="PSUM") as psB:
                for c in range(NCT):
                    po = psB.tile([128, H], F32, tag="po")
                    cs = slice(c * 128, (c + 1) * 128)
                    for i in range(NI):
                        for n in range(4):
                            ns = slice(n * 512, (n + 1) * 512)
                            nc.tensor.matmul(po[:, ns], ht_tiles[i][:, cs],
                                             wd_tiles[i][:, ns],
                                             start=(i == 0), stop=(i == NI - 1))
                    ot = op.tile([128, H], F32, tag="ot")
                    for n in range(4):
                        ns = slice(n * 512, (n + 1) * 512)
                        nc.vector.tensor_scalar_mul(ot[:, ns], po[:, ns],
                                                    cmb[:, c:c + 1])
                        nc.sync.dma_start(y_d[cs, ns], ot[:, ns])

    nc.compile()
    return nc


def _route(X: np.ndarray, Wr: np.ndarray):
    """Host router: top-2 of softmax(X @ Wr), renormalized over the top-2."""
    logits = X.astype(np.float64) @ Wr.astype(np.float64)
    order = np.argsort(-logits, axis=1)
    top1, top2 = order[:, 0], order[:, 1]
    rows = np.arange(T)
    l1, l2 = logits[rows, top1], logits[rows, top2]
    e21 = np.exp(l2 - l1)
    w1 = 1.0 / (1.0 + e21)
    w2 = e21 / (1.0 + e21)
    return top1, top2, w1.astype(np.float32), w2.astype(np.float32)


def _reference_numpy(hidden_states, w_router, w_gate, w_up, w_down):
    X = np.asarray(hidden_states, np.float32)
    top1, top2, w1, w2 = _route(X, np.asarray(w_router, np.float32))
    out = np.zeros((T, H), np.float32)
    for e in range(E):
        sel = np.where((top1 == e) | (top2 == e))[0]
        if len(sel) == 0:
            continue
        w = np.where(top1[sel] == e, w1[sel], w2[sel])[:, None]
        x = X[sel]
        h = (x @ w_gate[e])
        h = (h / (1.0 + np.exp(-h))) * (x @ w_up[e]) * w
        out[sel] += h @ w_down[e]
    return out


def kernel(hidden_states, w_router, w_gate, w_up, w_down):
    X = np.ascontiguousarray(hidden_states, dtype=np.float32)
    Wr = np.ascontiguousarray(w_router, dtype=np.float32)
    Wg = np.ascontiguousarray(w_gate, dtype=np.float32)
    Wu = np.ascontiguousarray(w_up, dtype=np.float32)
    Wd = np.ascontiguousarray(w_down, dtype=np.float32)

    top1, top2, w1, w2 = _route(X, Wr)
    sels, wts = [], []
    for e in range(E):
        sel = np.where((top1 == e) | (top2 == e))[0]
        sels.append(sel)
        wts.append(np.where(top1[sel] == e, w1[sel], w2[sel]))
    if max(len(s) for s in sels) > C:
        # Capacity overflow (cannot happen for the reference input
        # distribution); fall back to a host implementation.
        return _reference_numpy(X, Wr, Wg, Wu, Wd)

    if "nc" not in _CACHE:
        _CACHE["nc"] = _build()
    nc = _CACHE["nc"]

    Xr = _round_f32r(X)
    in_maps = []
    for e in range(E):
        sel, w = sels[e], wts[e]
        n = len(sel)
        xt = np.zeros((H, C), np.float32)
        xt[:, :n] = Xr[sel].T
        cmb = np.zeros((C, 1), np.float32)
        cmb[:n, 0] = w
        wg_sw = (_round_f32r(Wg[e]).reshape(NT, 128, NI, 128)
                 .transpose(2, 1, 0, 3).reshape(NI, 128, H))
        wu_sw = (_round_f32r(Wu[e]).reshape(NT, 128, NI, 128)
                 .transpose(2, 1, 0, 3).reshape(NI, 128, H))
        wd_r = _round_f32r(Wd[e])
        in_maps.append({
            "xt": np.ascontiguousarray(xt),
            "wg": np.ascontiguousarray(wg_sw),
            "wu": np.ascontiguousarray(wu_sw),
            "wd": wd_r,
            "cmb": cmb,
        })

    res = bass_utils.run_bass_kernel_spmd(nc, in_maps, list(range(E)))

    out = np.zeros((T, H), np.float32)
    for e in range(E):
        sel = sels[e]
        out[sel] += res.results[e]["y"][:len(sel)]
    return out
